# revision 16
# baseline (speedup 1.0000x reference)
"""Trainium2 Bass kernel for the ASBIGCN segment_reduce problem.

Contract: kernel(**inputs) takes the FULL unsharded inputs (as produced by the
problem's setup_inputs) and returns the FULL [64, 70000] float32 output.

Strategy (8 NeuronCores):
  - Batch-parallel over B=64 (8 items per core) for the K=3 transformer/GCN/
    biaffine stack. Activations live in SBUF transposed as [600, 8*256] f32r.
  - Per-item span-sum pooling -> per-core feature block [3000, 8].
  - Device AllGather of the tiny feature matrix, then tensor-parallel FC:
    each core computes [64, 8750] against its column slice of fc_w.
  - Host concatenates the 8 output slices into [64, 70000].

All matmuls run in float32r (full-rate PE mode, ~1e-4 relative error).
"""

import sys

sys.path.insert(0, "/opt/trn_rl_repo")

import math

import numpy as np

import concourse.bass as bass
import concourse.mybir as mybir
import concourse.tile as tile
from concourse import bacc
from concourse.bass_utils import run_bass_kernel_spmd
from concourse.masks import make_identity

F32 = mybir.dt.float32
F32R = mybir.dt.float32r
AX = mybir.AxisListType.X
EXP = mybir.ActivationFunctionType.Exp
RELU = mybir.ActivationFunctionType.Relu
ABS = mybir.ActivationFunctionType.Abs

NCORES = 8
B, S, D = 64, 256, 600
K = 3
BL = B // NCORES          # items per core
NS = BL * S               # 2048 batched free dim
DT, DP = 5, 120           # d split into 5 tiles of 120
OUT1 = 70000
OSH = OUT1 // NCORES      # 8750 output features per core
FDIM = 5 * D              # 3000
FT, FP = 25, 120          # feature tiles
OCH = 512                 # fc output chunk
NOC = math.ceil(OSH / OCH)

# per-tag double-buffer knobs
QK_BUFS = 2
WORK_BUFS = 2
STOP_AFTER = "all"  # debug: attn | ffc | graph | all
NO_CC = False       # debug: replace AllGather with local copy (for TimelineSim)


def _och(i):
    return min(OCH, OSH - i * OCH)


def build_nc():
    nc = bacc.Bacc("TRN2", target_bir_lowering=False, debug=False,
                   num_devices=NCORES)

    # ---------------- DRAM I/O ----------------
    xt0 = nc.dram_tensor("xt0", [D, NS], F32R, kind="ExternalInput")
    gts = nc.dram_tensor("gts", [BL, S, S], F32R, kind="ExternalInput")
    negmask = nc.dram_tensor("negmask", [BL, 1, S], F32, kind="ExternalInput")
    maskq = nc.dram_tensor("maskq", [2, 128, BL], F32, kind="ExternalInput")
    wspan = nc.dram_tensor("wspan", [BL, 1, S], F32R, kind="ExternalInput")
    houtT = nc.dram_tensor("houtT", [D, BL], F32R, kind="ExternalInput")
    wq = nc.dram_tensor("wq", [K, D, D], F32R, kind="ExternalInput")
    wk = nc.dram_tensor("wk", [K, D, D], F32R, kind="ExternalInput")
    wv = nc.dram_tensor("wv", [K, D, D], F32R, kind="ExternalInput")
    wo = nc.dram_tensor("wo", [K, D, D], F32R, kind="ExternalInput")
    wffc = nc.dram_tensor("wffc", [D, D], F32R, kind="ExternalInput")
    wlin = nc.dram_tensor("wlin", [D, D], F32R, kind="ExternalInput")
    wbiaff = nc.dram_tensor("wbiaff", [D, D], F32R, kind="ExternalInput")
    ffcb = nc.dram_tensor("ffcb", [D, 1], F32, kind="ExternalInput")
    fcw = nc.dram_tensor("fcw", [FDIM, OSH], F32R, kind="ExternalInput")
    fcb = nc.dram_tensor("fcb", [1, OSH], F32R, kind="ExternalInput")
    out = nc.dram_tensor("out", [B, OSH], F32, kind="ExternalOutput")

    with tile.TileContext(nc) as tc:
        with (
            tc.tile_pool(name="pers", bufs=1) as pers,
            tc.tile_pool(name="fcpers", bufs=1) as fpers,
            tc.tile_pool(name="psum", bufs=5, space="PSUM") as ps256,
            tc.tile_pool(name="psum512", bufs=2, space="PSUM") as ps512,
            tc.tile_pool(name="psumT", bufs=1, space="PSUM") as psT,
            tc.tile_pool(name="dram", bufs=1, space="DRAM") as dpool,
        ):
            # ---------------- persistent tiles ----------------
            Xt = [pers.tile([DP, NS], F32R, tag=f"Xt{d}", name=f"Xt{d}") for d in range(DT)]
            Xg = [pers.tile([DP, NS], F32R, tag=f"Xg{d}", name=f"Xg{d}") for d in range(DT)]
            for d in range(DT):
                nc.sync.dma_start(Xt[d][:], xt0[d * DP:(d + 1) * DP, :])
                nc.sync.dma_start(Xg[d][:], xt0[d * DP:(d + 1) * DP, :])

            identF = pers.tile([128, 128], F32, tag="identF")
            make_identity(nc, identF[:])
            identR = pers.tile([128, 128], F32R, tag="identR")
            nc.vector.tensor_copy(identR[:], identF[:])

            ffcb_c = [pers.tile([DP, 1], F32, tag=f"ffcb{d}", name=f"ffcb{d}") for d in range(DT)]
            for d in range(DT):
                nc.sync.dma_start(ffcb_c[d][:], ffcb[d * DP:(d + 1) * DP, :])

            mq_t = [pers.tile([128, BL], F32, tag=f"mqt{qt}", name=f"mqt{qt}")
                    for qt in range(2)]
            for qt in range(2):
                nc.sync.dma_start(mq_t[qt][:], maskq.ap()[qt])

            # fc-phase persistents (allocated below the stack pools)
            tmpc = [fpers.tile([DP, BL], F32, tag=f"tmpc{d}", name=f"tmpc{d}")
                    for d in range(DT)]
            tmp1c = [fpers.tile([DP, BL], F32, tag=f"tmp1c{d}", name=f"tmp1c{d}")
                     for d in range(DT)]
            FA = [fpers.tile([FP, BL], F32R, tag=f"FA{i}", name=f"FA{i}") for i in range(FT)]
            fT = [fpers.tile([FP, NCORES, BL], F32R, tag=f"fT{i}", name=f"fT{i}")
                  for i in range(FT)]
            ones32 = fpers.tile([1, B], F32, tag="ones32")
            nc.vector.memset(ones32[:], 1.0)
            ones = fpers.tile([1, B], F32R, tag="ones")
            nc.vector.tensor_copy(ones[:], ones32[:])
            with (
                tc.tile_pool(name="wattn", bufs=1) as wpool,
                tc.tile_pool(name="wrot", bufs=2) as wrot,
                tc.tile_pool(name="work", bufs=WORK_BUFS) as wk_pool,
            ):
                def load_w(pool, src, l=None, tagp=None):
                    """Load a [D, D] pre-transposed weight as DT tiles."""
                    tiles = []
                    for d in range(DT):
                        tg = f"{tagp or src.name}{d}"
                        t = pool.tile([DP, D], F32R, tag=tg, name=tg)
                        ap = src.ap()[l] if l is not None else src.ap()
                        nc.sync.dma_start(t[:], ap[d * DP:(d + 1) * DP, :])
                        tiles.append(t)
                    return tiles

                # ---------------- the 3-layer stack ----------------
                for l in range(K):
                    wq_sb = load_w(wpool, wq, l)
                    wk_sb = load_w(wpool, wk, l)
                    wv_sb = load_w(wpool, wv, l)
                    wo_sb = load_w(wpool, wo, l)
                    wffc_sb = load_w(wrot, wffc, tagp="wrot")

                    # ---- attention (per item) ----
                    for j in range(BL):
                        cols = slice(j * S, (j + 1) * S)
                        qT, kT = [], []
                        for (wmat, acc, nm) in ((wq_sb, qT, "q"),
                                                (wk_sb, kT, "k")):
                            for do in range(DT):
                                ps = ps256.tile([DP, S], F32, tag="ps256")
                                for di in range(DT):
                                    nc.tensor.matmul(
                                        ps[:],
                                        wmat[di][:, do * DP:(do + 1) * DP],
                                        Xt[di][:, cols],
                                        start=(di == 0), stop=(di == DT - 1))
                                t = wk_pool.tile([DP, S], F32R,
                                                 tag=f"{nm}T{do}",
                                                 name=f"{nm}T{do}",
                                                 bufs=QK_BUFS)
                                nc.vector.tensor_copy(t[:], ps[:])
                                acc.append(t)
                        # v natural: [256, 600] as 2 tiles [128, 600]
                        v_sb = []
                        for st in range(2):
                            t = wk_pool.tile([128, D], F32R, tag=f"v{st}",
                                             name=f"v{st}")
                            scol = slice(j * S + st * 128,
                                         j * S + st * 128 + 128)
                            for nt in range(2):
                                ps = ps256.tile([128, 300], F32, tag="ps256")
                                for di in range(DT):
                                    nc.tensor.matmul(
                                        ps[:], Xt[di][:, scol],
                                        wv_sb[di][:, nt * 300:(nt + 1) * 300],
                                        start=(di == 0), stop=(di == DT - 1))
                                nc.vector.tensor_copy(
                                    t[:, nt * 300:(nt + 1) * 300], ps[:])
                            v_sb.append(t)
                        # scores + softmax + transpose -> aT
                        aT = [wk_pool.tile([128, S], F32R, tag=f"aT{kt}", name=f"aT{kt}")
                              for kt in range(2)]
                        for qt in range(2):
                            ps = ps256.tile([128, S], F32, tag="ps256")
                            for di in range(DT):
                                nc.tensor.matmul(
                                    ps[:], qT[di][:, qt * 128:(qt + 1) * 128],
                                    kT[di][:],
                                    start=(di == 0), stop=(di == DT - 1))
                            mx = wk_pool.tile([128, 1], F32, tag="mx",
                                              name="mx")
                            nc.vector.reduce_max(mx[:], ps[:], axis=AX)
                            ngm = wk_pool.tile([128, 1], F32, tag="ngm",
                                               name="ngm")
                            nc.scalar.mul(ngm[:], mx[:], -1.0)
                            probs = wk_pool.tile([128, S], F32R, tag="probs",
                                                 name="probs", bufs=2)
                            Z = wk_pool.tile([128, 1], F32, tag="Z", name="Z")
                            nc.scalar.activation(probs[:], ps[:], EXP,
                                                 bias=ngm[:], scale=1.0,
                                                 accum_out=Z[:])
                            r = wk_pool.tile([128, 1], F32, tag="r", name="r")
                            nc.vector.reciprocal(r[:], Z[:])
                            nc.vector.tensor_scalar_mul(probs[:], probs[:],
                                                        r[:])
                            for kt in range(2):
                                pt_ps = psT.tile([128, 128], F32R, tag="psT")
                                nc.tensor.transpose(
                                    pt_ps[:],
                                    probs[:, kt * 128:(kt + 1) * 128],
                                    identR[:])
                                nc.vector.tensor_copy(
                                    aT[kt][:, qt * 128:(qt + 1) * 128],
                                    pt_ps[:])
                        # attnT = v^T a^T : [600, 256]
                        attnT = []
                        for d in range(DT):
                            ps = ps256.tile([DP, S], F32, tag="ps256")
                            for kt in range(2):
                                nc.tensor.matmul(
                                    ps[:], v_sb[kt][:, d * DP:(d + 1) * DP],
                                    aT[kt][:], start=(kt == 0), stop=(kt == 1))
                            t = wk_pool.tile([DP, S], F32R, tag=f"attnT{d}",
                                             name=f"attnT{d}")
                            nc.vector.tensor_copy(t[:], ps[:])
                            attnT.append(t)
                        # out proj + residual into Xt
                        for do in range(DT):
                            ps = ps256.tile([DP, S], F32, tag="ps256")
                            for di in range(DT):
                                nc.tensor.matmul(
                                    ps[:], wo_sb[di][:, do * DP:(do + 1) * DP],
                                    attnT[di][:],
                                    start=(di == 0), stop=(di == DT - 1))
                            nc.vector.tensor_add(Xt[do][:, cols],
                                                 Xt[do][:, cols], ps[:])

                    if STOP_AFTER == "attn":
                        continue
                    # ---- ffc (per item; all reads precede in-place write) ----
                    for j in range(BL):
                        ccol = slice(j * S, (j + 1) * S)
                        pss = []
                        for do in range(DT):
                            ps = ps256.tile([DP, S], F32, tag="ps256")
                            for di in range(DT):
                                nc.tensor.matmul(
                                    ps[:],
                                    wffc_sb[di][:, do * DP:(do + 1) * DP],
                                    Xt[di][:, ccol],
                                    start=(di == 0), stop=(di == DT - 1))
                            pss.append(ps)
                        for do in range(DT):
                            nc.vector.tensor_scalar_add(pss[do][:], pss[do][:],
                                                        ffcb_c[do][:])
                            nc.vector.tensor_add(Xt[do][:, ccol],
                                                 Xt[do][:, ccol], pss[do][:])

                    if STOP_AFTER == "ffc":
                        continue
                    # ---- graph conv (per item): Xg += relu((G/den) @ te) ----
                    wlin_sb = load_w(wrot, wlin, tagp="wrot")
                    for j in range(BL):
                        cols = slice(j * S, (j + 1) * S)
                        te_sb = []
                        for st in range(2):
                            t = wk_pool.tile([128, D], F32R, tag=f"te{st}",
                                             name=f"te{st}")
                            scol = slice(j * S + st * 128,
                                         j * S + st * 128 + 128)
                            for nt in range(2):
                                ps = ps256.tile([128, 300], F32, tag="ps256")
                                for di in range(DT):
                                    nc.tensor.matmul(
                                        ps[:], Xg[di][:, scol],
                                        wlin_sb[di][:, nt * 300:(nt + 1) * 300],
                                        start=(di == 0), stop=(di == DT - 1))
                                nc.vector.tensor_copy(
                                    t[:, nt * 300:(nt + 1) * 300], ps[:])
                            te_sb.append(t)
                        g_sb = []
                        for kt in range(2):
                            t = wk_pool.tile([128, S], F32R, tag=f"g{kt}",
                                             name=f"g{kt}")
                            nc.sync.dma_start(
                                t[:], gts.ap()[j, kt * 128:(kt + 1) * 128, :])
                            g_sb.append(t)
                        for d in range(DT):
                            ps = ps256.tile([DP, S], F32, tag="ps256")
                            for kt in range(2):
                                nc.tensor.matmul(
                                    ps[:], te_sb[kt][:, d * DP:(d + 1) * DP],
                                    g_sb[kt][:], start=(kt == 0),
                                    stop=(kt == 1))
                            rl = wk_pool.tile([DP, S], F32, tag="probs",
                                              name="rl", bufs=2)
                            nc.scalar.activation(rl[:], ps[:], RELU)
                            nc.vector.tensor_add(Xg[d][:, cols],
                                                 Xg[d][:, cols], rl[:])

                    if STOP_AFTER == "graph":
                        continue
                    # ---- mutual biaffine (per item) ----
                    wb_sb = load_w(wrot, wbiaff, tagp="wrot")
                    for j in range(BL):
                        cols = slice(j * S, (j + 1) * S)
                        nm_bc = wk_pool.tile([128, S], F32, tag="nmbc",
                                             name="nmbc")
                        nc.sync.dma_start(
                            nm_bc[:], negmask.ap()[j].partition_broadcast(128))
                        ptT, pgT = [], []
                        for (xsrc, acc, nm) in ((Xt, ptT, "q"),
                                                (Xg, pgT, "k")):
                            for do in range(DT):
                                ps = ps256.tile([DP, S], F32, tag="ps256")
                                for di in range(DT):
                                    nc.tensor.matmul(
                                        ps[:],
                                        wb_sb[di][:, do * DP:(do + 1) * DP],
                                        xsrc[di][:, cols],
                                        start=(di == 0), stop=(di == DT - 1))
                                t = wk_pool.tile([DP, S], F32R,
                                                 tag=f"{nm}T{do}",
                                                 name=f"{nm}T{do}",
                                                 bufs=QK_BUFS)
                                nc.vector.tensor_copy(t[:], ps[:])
                                acc.append(t)
                        # l1 = softmax(pt @ out_g^T + neg) ; l2 likewise
                        lT = {}
                        for (pT, xrhs, nm) in ((ptT, Xg, "l1"),
                                               (pgT, Xt, "l2")):
                            lT[nm] = [wk_pool.tile([128, S], F32R, tag=f"{nm}T{kt}", name=f"{nm}T{kt}")
                                      for kt in range(2)]
                            for qt in range(2):
                                ps = ps256.tile([128, S], F32, tag="ps256")
                                for di in range(DT):
                                    nc.tensor.matmul(
                                        ps[:],
                                        pT[di][:, qt * 128:(qt + 1) * 128],
                                        xrhs[di][:, cols],
                                        start=(di == 0), stop=(di == DT - 1))
                                probs = wk_pool.tile([128, S], F32R,
                                                     tag="probs",
                                                     name="probs", bufs=2)
                                nc.vector.tensor_add(probs[:], ps[:],
                                                     nm_bc[:])
                                mx = wk_pool.tile([128, 1], F32, tag="mx",
                                                  name="mx")
                                nc.vector.reduce_max(mx[:], probs[:], axis=AX)
                                ngm = wk_pool.tile([128, 1], F32, tag="ngm",
                                                   name="ngm")
                                nc.scalar.mul(ngm[:], mx[:], -1.0)
                                Z = wk_pool.tile([128, 1], F32, tag="Z",
                                                 name="Z")
                                nc.scalar.activation(probs[:], probs[:], EXP,
                                                     bias=ngm[:], scale=1.0,
                                                     accum_out=Z[:])
                                r = wk_pool.tile([128, 1], F32, tag="r",
                                                 name="r")
                                nc.vector.reciprocal(r[:], Z[:])
                                rm = wk_pool.tile([128, 1], F32, tag="rm",
                                                  name="rm")
                                nc.vector.tensor_mul(rm[:], r[:],
                                                     mq_t[qt][:, j:j + 1])
                                nc.vector.tensor_scalar_mul(probs[:],
                                                            probs[:], rm[:])
                                for kt in range(2):
                                    pt_ps = psT.tile([128, 128], F32R,
                                                     tag="psT")
                                    nc.tensor.transpose(
                                        pt_ps[:],
                                        probs[:, kt * 128:(kt + 1) * 128],
                                        identR[:])
                                    nc.vector.tensor_copy(
                                        lT[nm][kt][:, qt * 128:(qt + 1) * 128],
                                        pt_ps[:])
                        # natural-layout copies of Xt'' and Xg' for this item
                        natXt = [wk_pool.tile([128, D], F32R, tag=f"natXt{st}", name=f"natXt{st}")
                                 for st in range(2)]
                        natXg = [wk_pool.tile([128, D], F32R, tag=f"natXg{st}", name=f"natXg{st}")
                                 for st in range(2)]
                        for (X, nat) in ((Xt, natXt), (Xg, natXg)):
                            for st in range(2):
                                scol = slice(j * S + st * 128,
                                             j * S + st * 128 + 128)
                                for d in range(DT):
                                    pt_ps = psT.tile([128, 128], F32R,
                                                     tag="psT")
                                    nc.tensor.transpose(
                                        pt_ps[:, :DP], X[d][:, scol],
                                        identR[:DP, :DP])
                                    nc.vector.tensor_copy(
                                        nat[st][:, d * DP:(d + 1) * DP],
                                        pt_ps[:, :DP])
                        # o1 into Xt, o2 into Xg (mask folded into rm)
                        for (nat, lname, X) in ((natXg, "l1", Xt),
                                                (natXt, "l2", Xg)):
                            for d in range(DT):
                                ps = ps256.tile([DP, S], F32, tag="ps256")
                                for kt in range(2):
                                    nc.tensor.matmul(
                                        ps[:],
                                        nat[kt][:, d * DP:(d + 1) * DP],
                                        lT[lname][kt][:],
                                        start=(kt == 0), stop=(kt == 1))
                                nc.vector.tensor_add(X[d][:, cols],
                                                     X[d][:, cols], ps[:])

                # ------------- span sums + feature assembly -------------
                for j in range(BL):
                    cols = slice(j * S, (j + 1) * S)
                    ws_bc = wk_pool.tile([128, S], F32R, tag="nmbc",
                                         name="ws_bc")
                    nc.sync.dma_start(
                        ws_bc[:], wspan.ap()[j].partition_broadcast(128))
                    for d in range(DT):
                        msel = wk_pool.tile([DP, S], F32, tag="msel",
                                            name="msel")
                        nc.vector.tensor_mul(msel[:], Xt[d][:, cols],
                                             ws_bc[:DP, :])
                        nc.vector.reduce_sum(tmpc[d][:, j:j + 1], msel[:],
                                             axis=AX)
                        x0 = wk_pool.tile([DP, S], F32R, tag="x0", name="x0")
                        nc.sync.dma_start(x0[:],
                                          xt0[d * DP:(d + 1) * DP, cols])
                        nc.vector.tensor_mul(msel[:], x0[:], ws_bc[:DP, :])
                        nc.vector.reduce_sum(tmp1c[d][:, j:j + 1], msel[:],
                                             axis=AX)

                # feature blocks: [hout, tmp, tmp1, tmp*tmp1, |tmp-tmp1|]
                for d in range(DT):
                    nc.gpsimd.dma_start(FA[d][:], houtT[d * DP:(d + 1) * DP, :])
                    nc.vector.tensor_copy(FA[5 + d][:], tmpc[d][:])
                    nc.vector.tensor_copy(FA[10 + d][:], tmp1c[d][:])
                    nc.vector.tensor_mul(FA[15 + d][:], tmpc[d][:],
                                         tmp1c[d][:])
                    df = wk_pool.tile([DP, BL], F32, tag="msel", name="df")
                    nc.vector.tensor_sub(df[:], tmpc[d][:], tmp1c[d][:])
                    nc.scalar.activation(FA[20 + d][:], df[:], ABS)

                # ---------------- allgather features ----------------
                feat_l = dpool.tile([FT, FP, BL], F32R)
                for i in range(FT):
                    nc.sync.dma_start(feat_l[i], FA[i][:])
                feat_g = dpool.tile([NCORES, FT, FP, BL], F32R)
                if NO_CC:
                    nc.sync.dma_start(feat_g[0], feat_l[:])
                else:
                    nc.gpsimd.collective_compute(
                        "AllGather", mybir.AluOpType.bypass,
                        replica_groups=[list(range(NCORES))],
                        ins=[feat_l.opt()], outs=[feat_g.opt()])

                gview = feat_g[:].rearrange("c f p j -> f p c j")
                for i in range(FT):
                    nc.sync.dma_start(fT[i][:], gview[i])

            # ---------------- FC: out = feat @ fc_w.T + fc_b ----------------
            with tc.tile_pool(name="fc", bufs=2) as fcp:
                fcw_v = fcw.ap().rearrange("(f p) o -> p f o", p=FP)
                for oc in range(NOC):
                    w = _och(oc)
                    wg = fcp.tile([FP, FT, OCH], F32R, tag="wg", name="wg")
                    nc.sync.dma_start(wg[:, :, :w],
                                      fcw_v[:, :, oc * OCH:oc * OCH + w])
                    fcbc = fcp.tile([1, OCH], F32R, tag="fcbc", name="fcbc")
                    nc.sync.dma_start(fcbc[:, :w], fcb[:, oc * OCH:oc * OCH + w])
                    ps = ps512.tile([B, OCH], F32, tag="ps512")
                    for i in range(FT):
                        nc.tensor.matmul(
                            ps[:, :w],
                            fT[i][:].rearrange("p c j -> p (c j)"),
                            wg[:, i, :w], start=(i == 0), stop=False)
                    nc.tensor.matmul(ps[:, :w], ones[:], fcbc[:, :w],
                                     start=False, stop=True)
                    ot = fcp.tile([B, OCH], F32, tag="ot", name="ot")
                    nc.vector.tensor_copy(ot[:, :w], ps[:, :w])
                    nc.sync.dma_start(out[:, oc * OCH:oc * OCH + w],
                                      ot[:, :w])

    nc.compile()
    return nc


def prep_inputs(lstm_out, hout, dependency_graph, attn_in, attn_out, ffc_w,
                ffc_b, lin_w, biaff_w, fc_w, fc_b, text_len, spans):
    """Host-side sharding + layout transforms. Returns per-core input maps."""
    f32 = np.float32
    lstm_out = np.asarray(lstm_out, dtype=f32)
    hout = np.asarray(hout, dtype=f32)
    G = np.asarray(dependency_graph, dtype=f32)
    attn_in = np.asarray(attn_in, dtype=f32)
    attn_out = np.asarray(attn_out, dtype=f32)
    fc_w = np.asarray(fc_w, dtype=f32)
    text_len = np.asarray(text_len)
    spans = np.asarray(spans)

    scale = 1.0 / math.sqrt(D)
    wq = np.ascontiguousarray(
        np.stack([attn_in[l, :D, :].T * scale for l in range(K)]))
    wk_ = np.ascontiguousarray(np.stack([attn_in[l, D:2 * D, :].T
                                         for l in range(K)]))
    wv = np.ascontiguousarray(np.stack([attn_in[l, 2 * D:, :].T
                                        for l in range(K)]))
    wo = np.ascontiguousarray(np.stack([attn_out[l].T for l in range(K)]))
    wffc = np.ascontiguousarray(np.asarray(ffc_w, dtype=f32).T)
    wlin = np.ascontiguousarray(np.asarray(lin_w, dtype=f32).T)
    wbiaff = np.ascontiguousarray(np.asarray(biaff_w, dtype=f32).T)
    ffcb = np.ascontiguousarray(np.asarray(ffc_b, dtype=f32).reshape(D, 1))
    fcb = np.asarray(fc_b, dtype=f32).reshape(1, OUT1)

    idx = np.arange(S)
    mask = (idx[None, :] < text_len[:, None].astype(np.int64)).astype(f32)
    negm = (-10000.0 * (1.0 - mask)).reshape(B, 1, S)
    maskq_h = mask.reshape(B, 2, 128)
    s0 = spans[:, 0, 0].astype(np.int64)[:, None]
    e0 = spans[:, 0, 1].astype(np.int64)[:, None]
    wsp = ((idx[None, :] >= s0) & (idx[None, :] < e0)).astype(f32)
    wsp = wsp.reshape(B, 1, S)

    denom = G.sum(axis=2, keepdims=True) + 1e-7
    GTs = np.ascontiguousarray((G / denom).transpose(0, 2, 1))

    in_maps = []
    for c in range(NCORES):
        bs = slice(c * BL, (c + 1) * BL)
        xt0 = np.ascontiguousarray(
            lstm_out[bs].transpose(2, 0, 1).reshape(D, NS))
        in_maps.append({
            "xt0": xt0,
            "gts": np.ascontiguousarray(GTs[bs]),
            "negmask": np.ascontiguousarray(negm[bs]),
            "maskq": np.ascontiguousarray(maskq_h[bs].transpose(1, 2, 0)),
            "wspan": np.ascontiguousarray(wsp[bs]),
            "houtT": np.ascontiguousarray(hout[bs].T),
            "wq": wq, "wk": wk_, "wv": wv, "wo": wo,
            "wffc": wffc, "wlin": wlin, "wbiaff": wbiaff, "ffcb": ffcb,
            "fcw": np.ascontiguousarray(fc_w[c * OSH:(c + 1) * OSH, :].T),
            "fcb": np.ascontiguousarray(fcb[:, c * OSH:(c + 1) * OSH]),
        })
    return in_maps


_NC = None


def get_nc():
    global _NC
    if _NC is None:
        _NC = build_nc()
    return _NC


def kernel(**inputs) -> np.ndarray:
    nc = get_nc()
    in_maps = prep_inputs(**inputs)
    res = run_bass_kernel_spmd(nc, in_maps, list(range(NCORES)))
    return np.concatenate([res.results[c]["out"] for c in range(NCORES)],
                          axis=1)
